# revision 8
# baseline (speedup 1.0000x reference)
"""Trainium2 Bass kernel for nn_BACKFLOW (batched backflow determinant).

Math (faithful to the reference):
    cols = first 32 column indices of nonzeros of (x == 1), row-major scan
    h    = tanh(x @ W1 + b1)                       [B, 4]
    h    = tanh(h @ W2 + b2)                       [B, 4]
    S    = tanh(einsum('bf,foe->boe', h, W3) + b3)[:, cols, :]   [B, 32, 32]
    out  = det(S)                                  [B]

Distribution: pure data parallel over the walker (batch) axis across 8
NeuronCores; the tiny MLP params and the selected W3/b3 slices (via `cols`)
are replicated to every core.

Device algorithm per core (4096 walkers, 4 chunks x 1024):
  * PE: transpose x tiles, W1/W2 matmuls (tanh fused on ScalarE with bias),
    then per 128-walker tile S = tanh(h2_aug^T @ C_aug) into SBUF laid out
    as [128 walkers(partitions) x tiles x 1024(matrix)].
  * VectorE: batched unblocked LU over all walkers in parallel via
    broadcast (stride-0) access patterns; clamped pivots + adjacent-row
    pivoting for stability; det = product of pivots (sign-corrected).
  * One final PE transpose emits dets as [32, 128] for a contiguous DMA out.
"""

import sys

if "/opt/trn_rl_repo" not in sys.path:
    sys.path.insert(0, "/opt/trn_rl_repo")

import numpy as np

NCORES = 8
B = 32768
O = 128          # orbitals
E = 32           # electrons == slater matrix size
H = 4            # MLP hidden
BC = B // NCORES     # walkers per core
NCHUNK = 4
CW = BC // NCHUNK    # walkers per chunk
NT = CW // 128       # 128-walker tiles per chunk
PIV_CLAMP = 1e-6
NEIGHBOR_PIVOT = True

_CACHE = {}


def _patch_tile_tail_drain():
    """The tail drain TileContext emits carries >1 sem wait; this walrus
    build only accepts one sync wait per TPB_CTRL drain.  Split them."""
    import concourse.mybir as mybir
    import concourse.tile as tile_mod
    from concourse.tile import TileContext

    if getattr(TileContext, "_drain_patched", False):
        return
    _ScopedClock = tile_mod.ScopedClock

    def _patched(self, tick_clock, wait_clock):
        drain_inst = self.nc.sync.drain()
        wait_clock.add_sem_waits(
            drain_inst.ins, _ScopedClock({None: tick_clock.global_clock})
        )
        si = drain_inst.ins.sync_info
        if si is not None and len(si.on_wait) > 1:
            waits = list(si.on_wait)
            drain_inst.ins.sync_info = mybir.SyncInfo(
                on_wait=waits[:1], on_update=list(si.on_update)
            )
            for i in range(1, len(waits)):
                d2 = self.nc.sync.drain()
                d2.ins.sync_info = mybir.SyncInfo(on_wait=[waits[i]], on_update=[])
        self.nc.all_engine_barrier()
        assert self.sems is not None
        popped = self.nc._tile_sem_poison_stack.pop()
        assert popped is self._sem_poison
        self.nc.clear_and_free_semaphores(list(self.sems.allocated().values()))
        self.nc.all_engine_barrier()

    TileContext._drain_and_barrier = _patched
    TileContext._drain_patched = True


def _split_multi_waits(nc):
    """This walrus build accepts at most one sync-wait command per TPB
    instruction.  Move surplus waits onto same-engine NOPs inserted right
    before the owning instruction."""
    import concourse.mybir as mybir

    count = 0
    for blk in nc.m.functions[0].blocks:
        insts = list(blk.instructions)
        out = []
        changed = False
        for inst in insts:
            si = inst.sync_info
            if si is not None and len(si.on_wait) > 1:
                waits = list(si.on_wait)
                for w in waits[:-1]:
                    count += 1
                    nop = mybir.InstNoOp(
                        name=f"Wsplit-{count}", engine=inst.engine
                    )
                    nop.sync_info = mybir.SyncInfo(on_wait=[w], on_update=[])
                    out.append(nop)
                inst.sync_info = mybir.SyncInfo(
                    on_wait=[waits[-1]], on_update=list(si.on_update)
                )
                changed = True
            out.append(inst)
        if changed:
            blk.instructions = out
    return count


def _build_bass():
    import concourse.bass as bass
    import concourse.mybir as mybir
    from concourse.masks import make_identity
    from concourse.tile import TileContext

    _patch_tile_tail_drain()

    f32 = mybir.dt.float32
    u32 = mybir.dt.uint32
    Alu = mybir.AluOpType
    Act = mybir.ActivationFunctionType

    nc = bass.Bass()
    xc = nc.dram_tensor("xc", [BC, O], f32, kind="ExternalInput")
    w1 = nc.dram_tensor("w1", [O, H], f32, kind="ExternalInput")
    w2 = nc.dram_tensor("w2", [H, H], f32, kind="ExternalInput")
    bias1 = nc.dram_tensor("bias1", [H, 1], f32, kind="ExternalInput")
    bias2 = nc.dram_tensor("bias2", [H, 1], f32, kind="ExternalInput")
    caug = nc.dram_tensor("caug", [H + 1, E * E], f32, kind="ExternalInput")
    out = nc.dram_tensor("out", [BC // 128, 128], f32, kind="ExternalOutput")

    with TileContext(nc) as tc:
        with (
            tc.tile_pool(name="consts", bufs=1) as consts,
            tc.tile_pool(name="mlp", bufs=2) as mlp,
            tc.tile_pool(name="apool", bufs=2) as apool,
            tc.tile_pool(name="work", bufs=1) as work,
            tc.tile_pool(name="ps_t", bufs=2, space="PSUM") as ps_t,
            tc.tile_pool(name="ps_m", bufs=2, space="PSUM") as ps_m,
        ):
            ident = consts.tile([128, 128], f32)
            make_identity(nc, ident)
            w1t = consts.tile([O, H], f32)
            nc.sync.dma_start(w1t, w1[:, :])
            w2t = consts.tile([H, H], f32)
            nc.sync.dma_start(w2t, w2[:, :])
            b1t = consts.tile([H, 1], f32)
            nc.sync.dma_start(b1t, bias1[:, :])
            b2t = consts.tile([H, 1], f32)
            nc.sync.dma_start(b2t, bias2[:, :])
            cgt = consts.tile([H, E * E], f32)
            nc.sync.dma_start(cgt, caug[0:H, :])
            b3r = consts.tile([1, E * E], f32)
            nc.sync.dma_start(b3r, caug[H : H + 1, :])
            onesr = consts.tile([1, 128], f32)
            nc.vector.memset(onesr, 1.0)

            detall = consts.tile([128, BC // 128], f32)

            # persistent LU scratch
            detc = work.tile([128, NT], f32)
            rcp = work.tile([128, NT], f32)
            pivs = work.tile([128, NT], f32)
            sb2 = work.tile([128, NT], u32)
            na0 = work.tile([128, NT], f32)
            na1 = work.tile([128, NT], f32)
            maskU = work.tile([128, NT], u32)
            maskF = work.tile([128, NT], f32)
            sflip = work.tile([128, NT], f32)
            rowp = work.tile([128, NT, E], f32)
            trow = work.tile([128, NT, E], f32)
            tmp = work.tile([128, NT, E - 1, E - 1], f32)

            for c in range(NCHUNK):
                # ---- load + transpose x ----
                xx = mlp.tile([128, NT, O], f32, tag="xx")
                nc.sync.dma_start(
                    xx,
                    xc[c * CW : (c + 1) * CW, :].rearrange(
                        "(t p) o -> p t o", p=128
                    ),
                )
                xT = mlp.tile([O, NT, 128], f32, tag="xT")
                for t in range(NT):
                    pst = ps_t.tile([128, 128], f32, tag="pst")
                    nc.tensor.transpose(pst, xx[:, t, :], ident)
                    nc.scalar.copy(xT[:, t, :], pst)

                # ---- h1 = tanh(W1^T xT + b1) : [4, CW] ----
                xTf = xT.rearrange("p t w -> p (t w)")
                h1 = mlp.tile([H, CW], f32, tag="h1")
                for s in range(CW // 512):
                    ph = ps_t.tile([H, 512], f32, tag="ph")
                    nc.tensor.matmul(ph, w1t, xTf[:, s * 512 : (s + 1) * 512])
                    nc.scalar.activation(
                        h1[:, s * 512 : (s + 1) * 512], ph, Act.Tanh, bias=b1t
                    )

                # ---- h2 = tanh(W2^T h1 + b2) : [4, CW] ----
                h2a = mlp.tile([H, CW], f32, tag="h2a")
                for s in range(CW // 512):
                    ph2 = ps_t.tile([H, 512], f32, tag="ph")
                    nc.tensor.matmul(ph2, w2t, h1[:, s * 512 : (s + 1) * 512])
                    nc.scalar.activation(
                        h2a[0:H, s * 512 : (s + 1) * 512], ph2, Act.Tanh, bias=b2t
                    )

                # ---- slater matrices: A[p, t, :] = tanh(h2aug_t^T @ Caug) ----
                A = apool.tile([128, NT, E * E], f32, tag="A")
                for t in range(NT):
                    pm = ps_m.tile([128, E * E], f32, tag="pm")
                    for s in range(2):
                        nc.tensor.matmul(
                            pm[:, s * 512 : (s + 1) * 512],
                            h2a[:, t * 128 : (t + 1) * 128],
                            cgt[:, s * 512 : (s + 1) * 512],
                            start=True,
                            stop=False,
                        )
                        nc.tensor.matmul(
                            pm[:, s * 512 : (s + 1) * 512],
                            onesr,
                            b3r[:, s * 512 : (s + 1) * 512],
                            start=False,
                            stop=True,
                        )
                    nc.scalar.activation(A[:, t, :], pm, Act.Tanh)

                # ---- batched LU (no transpose; walkers on partitions) ----
                A4 = A.rearrange("p t (i j) -> p t i j", i=E)
                nc.vector.memset(detc, 1.0)
                for k in range(E):
                    if NEIGHBOR_PIVOT and k < E - 1:
                        L = E - k
                        nc.vector.tensor_mul(na0, A4[:, :, k, k], A4[:, :, k, k])
                        nc.vector.tensor_mul(
                            na1, A4[:, :, k + 1, k], A4[:, :, k + 1, k]
                        )
                        nc.vector.tensor_tensor(maskU, na1, na0, Alu.is_gt)
                        nc.vector.tensor_tensor(maskF, na1, na0, Alu.is_gt)
                        mb = maskU[:, :, None].broadcast_to([128, NT, L])
                        rK = A4[:, :, k, k:]
                        rK1 = A4[:, :, k + 1, k:]
                        nc.scalar.copy(trow[:, :, :L], rK)
                        nc.vector.copy_predicated(rK, mb, rK1)
                        nc.vector.copy_predicated(rK1, mb, trow[:, :, :L])
                        nc.vector.tensor_scalar(
                            sflip, maskF, -2.0, 1.0, Alu.mult, Alu.add
                        )
                        nc.vector.tensor_mul(detc, detc, sflip)

                    piv = A4[:, :, k, k]
                    # pivs = sign(piv) * max(|piv|, clamp), via fp32 bit tricks
                    nc.vector.tensor_scalar(
                        pivs.bitcast(u32),
                        piv.bitcast(u32),
                        0x7FFFFFFF,
                        None,
                        Alu.bitwise_and,
                    )
                    nc.vector.tensor_scalar(pivs, pivs, PIV_CLAMP, None, Alu.max)
                    nc.vector.tensor_scalar(
                        sb2, piv.bitcast(u32), -0x80000000, None, Alu.bitwise_and
                    )
                    nc.vector.tensor_tensor(
                        pivs.bitcast(u32), pivs.bitcast(u32), sb2, Alu.bitwise_or
                    )
                    nc.vector.tensor_mul(detc, detc, pivs)

                    if k < E - 1:
                        n = E - 1 - k
                        nc.vector.reciprocal(rcp, pivs)
                        row = A4[:, :, k, k + 1 :]
                        col = A4[:, :, k + 1 :, k]
                        nc.vector.tensor_mul(
                            rowp[:, :, :n],
                            row,
                            rcp[:, :, None].broadcast_to([128, NT, n]),
                        )
                        nc.vector.tensor_mul(
                            tmp[:, :, :n, :n],
                            col[:, :, :, None].broadcast_to([128, NT, n, n]),
                            rowp[:, :, None, :n].broadcast_to([128, NT, n, n]),
                        )
                        nc.vector.tensor_sub(
                            A4[:, :, k + 1 :, k + 1 :],
                            A4[:, :, k + 1 :, k + 1 :],
                            tmp[:, :, :n, :n],
                        )

                nc.scalar.copy(detall[:, c * NT : (c + 1) * NT], detc)

            # ---- emit dets: [128, 32] -> [32, 128] -> DRAM ----
            psd = ps_t.tile([BC // 128, 128], f32, tag="ph")
            nc.tensor.transpose(psd, detall, ident)
            dsb = consts.tile([BC // 128, 128], f32)
            nc.scalar.copy(dsb, psd)
            nc.sync.dma_start(out[:, :], dsb)

    nsplit = _split_multi_waits(nc)
    if nsplit:
        print(f"[kernel] split {nsplit} surplus sync waits onto NOPs")
    return nc


def _get_nc():
    if "nc" not in _CACHE:
        _CACHE["nc"] = _build_bass()
    return _CACHE["nc"]


def _first_nonzero_cols(x: np.ndarray) -> np.ndarray:
    """First E column indices of nonzeros of (x == 1) in row-major order."""
    cols = []
    for r in range(x.shape[0]):
        nz = np.flatnonzero(x[r] == 1)
        take = min(E - len(cols), nz.size)
        if take:
            cols.extend(nz[:take].tolist())
        if len(cols) >= E:
            break
    return np.asarray(cols[:E], dtype=np.int64)


def kernel(x, W1, b1, W2, b2, W3, b3):
    from concourse import bass_utils

    x = np.ascontiguousarray(np.asarray(x, dtype=np.float32))
    W1 = np.asarray(W1, dtype=np.float32)
    b1 = np.asarray(b1, dtype=np.float32)
    W2 = np.asarray(W2, dtype=np.float32)
    b2 = np.asarray(b2, dtype=np.float32)
    W3 = np.asarray(W3, dtype=np.float32)
    b3 = np.asarray(b3, dtype=np.float32)

    cols = _first_nonzero_cols(x)
    csel = W3[:, cols, :].reshape(H, E * E)
    bsel = b3[cols, :].reshape(1, E * E)
    caug = np.ascontiguousarray(np.concatenate([csel, bsel], axis=0))

    shared = {
        "w1": W1,
        "w2": W2,
        "bias1": b1.reshape(H, 1),
        "bias2": b2.reshape(H, 1),
        "caug": caug,
    }
    in_maps = [
        {"xc": x[c * BC : (c + 1) * BC], **shared} for c in range(NCORES)
    ]

    nc = _get_nc()
    res = bass_utils.run_bass_kernel_spmd(nc, in_maps, core_ids=list(range(NCORES)))
    det = np.concatenate(
        [np.asarray(res.results[c]["out"]).reshape(BC) for c in range(NCORES)]
    )
    return det.astype(np.float32)
